# revision 32
# baseline (speedup 1.0000x reference)
"""Trainium2 Bass kernel for a dense-transformer attention block.

Reference semantics (T=2048, D=2048, 16 heads, d_h=128):
    h = RMSNorm(x) * ln_w
    q,k,v = h @ W{q,k,v}.T  -> (n_h, T, d_h);  RoPE(q, k)
    att = softmax(causal(q k^T / sqrt(d_h))) @ v
    out = x + att @ Wo.T          (attention_mask is all-ones per spec)

Distribution: head-parallel over 8 cores (2 heads/core).  Per 512-row block B:
  phase 1  QKV projections (bf16, contract over d_model); ln_w folded into the
           weights on the host; RMSNorm row scales r[t] enter via r-scaled RoPE
           tables (q,k) and fused per-row V scaling on ScalarE.
  phase 2  causal attention with scores TRANSPOSED (S^T[j,i]); softmax row-sums
           accumulate on the PE via a ones-vector matmul; exp on ScalarE;
           single-J score tiles with 2 PSUM slots so the PE never waits on exp.
  phase 3  one merged AllGather per block (both heads) of att^T rows.
  phase 4  output projection, interleaved into the block loop (P4(B-2) between
           phase 1 and phase 2 of block B) so the in-order PE stream never
           parks behind a collective wait.

Engine/queue discipline (collective waits block the posting engine!):
  sync   - forward HBM->SBUF loads only (weights, x, cos/sin, residual)
  vector - never-gated SBUF-source DMAs (r transposes, AG inputs, out stores)
  gpsimd - collective triggers + post-AG readbacks (allowed to block)
  partition broadcasts are rank-1 PE matmuls (ones column x row vector).
Host assembles out = concat(out_colsT.T, axis=1).
"""

import math

import numpy as np

EPS = 1e-5
NEG = -1.0e30

CFG_FULL = dict(T=2048, D=2048, n_cores=8, heads_per_core=2)


# --------------------------------------------------------------------------
# device program
# --------------------------------------------------------------------------
def build_nc(T, D, n_cores, heads_per_core):
    import concourse.mybir as mybir
    import concourse.tile as tile
    from concourse import bacc

    DH = 128                      # head dim (hard-wired into layout)
    P = 128                       # partitions
    NH = heads_per_core
    DL = NH * DH                  # local width (q/k/v columns per core)
    KC = D // P                   # k-chunks over d_model
    TB = T // 512                 # 512-wide t blocks
    NIB = T // 512                # 512-wide i blocks
    NTS = T // P                  # 128-wide t subtiles
    f32 = mybir.dt.float32
    bf16 = mybir.dt.bfloat16
    i32 = mybir.dt.int32

    nc = bacc.Bacc("TRN2", target_bir_lowering=False, debug=False,
                   num_devices=n_cores)

    # ---- I/O ----
    xT = nc.dram_tensor("xT", [D, T], bf16, kind="ExternalInput").ap()
    xct_in = nc.dram_tensor("x_colsT", [DL, T], f32, kind="ExternalInput").ap()
    # weight tensors arrive host-pretiled in SBUF layout [P, KC*DL]; ln_w is
    # folded into wq/wk/wv on the host
    wq_t = nc.dram_tensor("wq_t", [P, KC * DL], bf16, kind="ExternalInput").ap()
    wk_t = nc.dram_tensor("wk_t", [P, KC * DL], bf16, kind="ExternalInput").ap()
    wv_t = nc.dram_tensor("wv_t", [P, KC * DL], bf16, kind="ExternalInput").ap()
    wo_t = nc.dram_tensor("wo_t", [P, KC * DL], bf16, kind="ExternalInput").ap()
    cosT = nc.dram_tensor("cosT", [DH, T], f32, kind="ExternalInput").ap()
    sinT = nc.dram_tensor("sinT", [DH, T], f32, kind="ExternalInput").ap()
    rot_t = nc.dram_tensor("rot_t", [DH, DH], bf16, kind="ExternalInput").ap()
    out_cT = nc.dram_tensor("out_colsT", [DL, T], f32,
                            kind="ExternalOutput").ap()

    xTr = xT.rearrange("(kc p) t -> p kc t", p=P)
    xctr = xct_in.rearrange("(js p) t -> p js t", p=P)
    outr = out_cT.rearrange("(js p) t -> p js t", p=P)

    Act = mybir.ActivationFunctionType
    Alu = mybir.AluOpType
    inv_sqrt_dh = 1.0 / math.sqrt(DH)
    MAGIC = 0x5F3759DF

    with tile.TileContext(nc) as tc, \
            tc.tile_pool(name="persist", bufs=1) as persist:
        # ---------------- long-lived tensors ----------------
        Q_sb = persist.tile([P, NH, T], bf16, tag="Q_sb")
        K_sb = persist.tile([P, NH, T], bf16, tag="K_sb")
        V_sb = persist.tile([P, NTS, DL], bf16, tag="V_sb")
        rcol_sb = persist.tile([P, NTS], f32, tag="rcol_sb")
        rrow_sb = persist.tile([1, T], f32, tag="rrow_sb")
        ones_bf = persist.tile([P, 1], bf16, tag="ones_bf")
        ones_row = persist.tile([1, P], bf16, tag="ones_row")
        masks_sb = persist.tile([P, 4, 512], f32, tag="masks_sb")
        rot_sb = persist.tile([P, DH], bf16, tag="rot_sb")

        nc.sync.dma_start(rot_sb[:], rot_t)
        nc.vector.memset(ones_bf[:], 1.0)
        nc.vector.memset(ones_row[:], 1.0)
        warm_sb = persist.tile([P, 128], bf16, tag="warm_sb")
        nc.vector.memset(warm_sb[:], 0.0)
        nc.gpsimd.memset(masks_sb[:], 0.0)
        for r in range(4):
            # keep (0) where i - j >= 0 with i = 512*B + f, j = 128*J + p,
            # offset r = J - 4*B  ->  f - p - 128 r >= 0
            nc.gpsimd.affine_select(
                out=masks_sb[:, r, :], in_=masks_sb[:, r, :],
                pattern=[[1, 512]], channel_multiplier=-1, base=-128 * r,
                compare_op=Alu.is_ge, fill=NEG)

        with tc.tile_pool(name="dram", bufs=1, space="DRAM") as dram_pool:
            ag_shared = "Shared" if n_cores > 4 else "Local"
            ag_in = [dram_pool.tile([NH * DH, 512], bf16, tag=f"agi{b}",
                                    name=f"ag_in{b}")
                     for b in range(NIB)]
            ag_out = [dram_pool.tile([n_cores * NH * DH, 512], bf16,
                                     addr_space=ag_shared, tag=f"ago{b}",
                                     name=f"ag_out{b}")
                      for b in range(NIB)]

            # PE warmup: ~9us of back-to-back dummy matmuls so the HAM
            # clock gate opens (and stays open) until the first real
            # matmul's inputs have landed
            with tc.tile_pool(name="warm_ps", bufs=1, space="PSUM") as wmps:
                wps = wmps.tile([P, 128], f32, tag="wm")
                for _ in range(100):
                    nc.tensor.matmul(wps[:], warm_sb[:], warm_sb[:],
                                     start=True, stop=True)

            with (
                tc.tile_pool(name="wqkv", bufs=1) as wpool,
                tc.tile_pool(name="cs_raw", bufs=1) as cspool,
                tc.tile_pool(name="xk", bufs=2) as xkpool,
                tc.tile_pool(name="sq", bufs=4) as sqpool,
                tc.tile_pool(name="tmp1", bufs=2) as tmppool,
                tc.tile_pool(name="qs1", bufs=4) as qspool,
                tc.tile_pool(name="pt", bufs=4) as ptpool,
                tc.tile_pool(name="fin", bufs=1) as finpool,
                tc.tile_pool(name="ag_sb", bufs=2) as agpool,
                tc.tile_pool(name="xc", bufs=2) as xcpool,
                tc.tile_pool(name="osb", bufs=2) as opool,
                # PSUM: proj 3 + st 2 + acc 2 + ssum/bcast 1 = 8 banks
                tc.tile_pool(name="proj_ps", bufs=3, space="PSUM") as projps,
                tc.tile_pool(name="st_ps", bufs=2, space="PSUM") as stpool,
                tc.tile_pool(name="acc_ps", bufs=2, space="PSUM") as accpool,
                tc.tile_pool(name="ssum_ps", bufs=1, space="PSUM") as ssumps,
            ):
                wq_sb = wpool.tile([P, KC, DL], bf16, tag="wq")
                wk_sb = wpool.tile([P, KC, DL], bf16, tag="wk")
                wv_sb = wpool.tile([P, KC, DL], bf16, tag="wv")
                wo_sb = wpool.tile([P, KC, DL], bf16, tag="wo")
                cos_r = cspool.tile([P, T], f32, tag="cos")
                sin_r = cspool.tile([P, T], f32, tag="sin")

                # ---- initial loads, ordered so block 0 unblocks earliest ----
                xkb = [xkpool.tile([P, KC, 512], bf16, tag="xk",
                                   name=f"xkb{b}") for b in range(TB)]
                wqr = wq_t.rearrange("p (kc j) -> p kc j", j=DL)
                for kq in range(0, KC, 4):
                    nc.sync.dma_start(wq_sb[:, kq:kq + 4, :],
                                      wqr[:, kq:kq + 4, :])
                    nc.sync.dma_start(xkb[0][:, kq:kq + 4, :],
                                      xTr[:, kq:kq + 4, 0:512])
                nc.sync.dma_start(cos_r[:, 0:512], cosT[:, 0:512])
                nc.sync.dma_start(sin_r[:, 0:512], sinT[:, 0:512])
                nc.sync.dma_start(wk_sb[:], wk_t.rearrange("p (kc j) -> p kc j", j=DL))
                nc.sync.dma_start(wv_sb[:], wv_t.rearrange("p (kc j) -> p kc j", j=DL))
                nc.sync.dma_start(xkb[1][:], xTr[:, :, 512:1024])
                for B in range(1, TB):
                    tb = slice(512 * B, 512 * B + 512)
                    nc.sync.dma_start(cos_r[:, tb], cosT[:, tb])
                    nc.sync.dma_start(sin_r[:, tb], sinT[:, tb])
                nc.sync.dma_start(wo_sb[:], wo_t.rearrange("p (kc j) -> p kc j", j=DL))
                for B in range(2, TB):
                    tb = slice(512 * B, 512 * B + 512)
                    nc.sync.dma_start(xkb[B][:], xTr[:, :, tb])

                ag_tiles = {}   # B -> readback SBUF tile [P, KC, 512]
                xct_tiles = {}  # B -> residual tile [P, 2, 512]

                def phase1(B):
                    tb = slice(512 * B, 512 * B + 512)
                    xx = xkb[B]
                    # --- Q pass + x^2 row-sums, interleaved per kc ---
                    srow_t = projps.tile([P, 512], f32, tag="p", name=f"srow{B}")
                    srow = srow_t[0:1, :]
                    qp = [projps.tile([P, 512], f32, tag="p", name=f"qp{h}_{B}")
                          for h in range(NH)]
                    LAG = 3
                    sqs = {}

                    def srow_mm(kc):
                        nc.tensor.matmul(srow, ones_bf[:], sqs.pop(kc)[:],
                                         start=(kc == 0), stop=(kc == KC - 1))

                    for kc in range(KC):
                        sq = sqpool.tile([P, 512], bf16, tag="sq")
                        if kc % 2 == 0:
                            nc.vector.tensor_tensor(sq[:], xx[:, kc, :],
                                                    xx[:, kc, :], Alu.mult)
                        else:
                            nc.scalar.activation(sq[:], xx[:, kc, :],
                                                 Act.Square)
                        sqs[kc] = sq
                        if kc >= LAG:
                            srow_mm(kc - LAG)
                        for h in range(NH):
                            hs = slice(DH * h, DH * (h + 1))
                            nc.tensor.matmul(qp[h][:], wq_sb[:, kc, hs],
                                             xx[:, kc, :], start=(kc == 0),
                                             stop=(kc == KC - 1))
                    for kc in range(KC - LAG, KC):
                        srow_mm(kc)
                    # r = rsqrt(mean + eps): bit-trick seed + 2 Newton (DVE)
                    rr = rrow_sb[0:1, tb]
                    mrow = tmppool.tile([1, 512], f32, tag="mrow")
                    nc.vector.tensor_scalar(mrow[:], srow, 1.0 / D, EPS,
                                            Alu.mult, Alu.add)
                    for h in range(NH):
                        nc.scalar.activation(Q_sb[:, h, tb], qp[h][:], Act.Copy)
                    ri = tmppool.tile([1, 512], i32, tag="ri")
                    nc.vector.tensor_scalar(ri[:], mrow[:].bitcast(i32), 1, None,
                                            Alu.arith_shift_right)
                    nc.vector.tensor_scalar(ri[:], ri[:], -1, MAGIC,
                                            Alu.mult, Alu.add)
                    rrv = ri[:].bitcast(f32)
                    tn = tmppool.tile([1, 512], f32, tag="tn")
                    nc.vector.tensor_tensor(tn[:], rrv, rrv, Alu.mult)
                    nc.vector.tensor_tensor(tn[:], tn[:], mrow[:], Alu.mult)
                    nc.vector.tensor_scalar(tn[:], tn[:], -0.5, 1.5,
                                            Alu.mult, Alu.add)
                    nc.vector.tensor_tensor(rrv, rrv, tn[:], Alu.mult)
                    nc.vector.tensor_tensor(tn[:], rrv, rrv, Alu.mult)
                    nc.vector.tensor_tensor(tn[:], tn[:], mrow[:], Alu.mult)
                    nc.vector.tensor_scalar(tn[:], tn[:], -0.5, 1.5,
                                            Alu.mult, Alu.add)
                    nc.vector.tensor_tensor(rr, rrv, tn[:], Alu.mult)
                    for s in range(4):
                        i = 4 * B + s
                        nc.scalar.dma_start(
                            out=rcol_sb[:, i:i + 1],
                            in_=rrow_sb[0:1, 512 * B + 128 * s:
                                        512 * B + 128 * (s + 1)])
                    # --- K pass ---
                    kp = [projps.tile([P, 512], f32, tag="p", name=f"kp{h}_{B}")
                          for h in range(NH)]
                    for kc in range(KC):
                        for h in range(NH):
                            hs = slice(DH * h, DH * (h + 1))
                            nc.tensor.matmul(kp[h][:], wk_sb[:, kc, hs],
                                             xx[:, kc, :], start=(kc == 0),
                                             stop=(kc == KC - 1))
                    for h in range(NH):
                        nc.scalar.activation(K_sb[:, h, tb], kp[h][:], Act.Copy)
                    # broadcast r across partitions: rank-1 PE matmul (bf16
                    # operands -- fp32 matmuls run as slow 2-pass LOW/HIGH)
                    rrb = tmppool.tile([1, 512], bf16, tag="rrb")
                    nc.vector.tensor_copy(rrb[:], rr)
                    rbc = ssumps.tile([P, 512], f32, tag="ss", name=f"rbc{B}")
                    nc.tensor.matmul(rbc[:], ones_row[:], rrb[:],
                                     start=True, stop=True)
                    nc.vector.tensor_tensor(cos_r[:, tb], cos_r[:, tb], rbc[:], Alu.mult)
                    nc.vector.tensor_tensor(sin_r[:, tb], sin_r[:, tb], rbc[:], Alu.mult)
                    # --- V pass, one 128-row subtile at a time; the r scale
                    #     rides along on the ScalarE evacuation ---
                    for ts in range(4):
                        i = 4 * B + ts
                        vp = projps.tile([P, 512], f32, tag="p", name=f"vp{i}")
                        for kc in range(KC):
                            nc.tensor.matmul(vp[:, :DL],
                                             xx[:, kc, P * ts:P * (ts + 1)],
                                             wv_sb[:, kc, :], start=(kc == 0),
                                             stop=(kc == KC - 1))
                        nc.scalar.activation(V_sb[:, i, :], vp[:, :DL],
                                             Act.Copy, scale=rcol_sb[:, i:i + 1])
                    # --- RoPE in place (r enters via the scaled tables) ---
                    qs = []
                    for buf in (Q_sb, K_sb):
                        for h in range(NH):
                            q = qspool.tile([P, 512], bf16, tag="qs")
                            nc.vector.tensor_tensor(q[:], buf[:, h, tb],
                                                    sin_r[:, tb], Alu.mult)
                            qs.append(q)
                    rps = []
                    for i in range(4):
                        rp = projps.tile([P, 512], f32, tag="p", name=f"rp{i}_{B}")
                        nc.tensor.matmul(rp[:], rot_sb[:], qs[i][:],
                                         start=True, stop=True)
                        rps.append(rp)
                    i = 0
                    for buf in (Q_sb, K_sb):
                        for h in range(NH):
                            nc.vector.tensor_tensor(buf[:, h, tb], buf[:, h, tb],
                                                    cos_r[:, tb], Alu.mult)
                            nc.vector.tensor_tensor(buf[:, h, tb], buf[:, h, tb],
                                                    rps[i][:], Alu.add)
                            i += 1

                def phase2(B):
                    ib = slice(512 * B, 512 * B + 512)
                    Jmax = 4 * B + 3
                    for h in range(NH):
                        hs = slice(DH * h, DH * (h + 1))
                        av = accpool.tile([P, 512], f32, tag="acc",
                                          name=f"av{h}_{B}")
                        ss_t = ssumps.tile([P, 512], f32, tag="ss",
                                           name=f"ss{h}_{B}")
                        ssum = ss_t[0:1, :]
                        pts = {}
                        # diagonal (masked) tiles first: their extra DVE
                        # mask hop pipelines under the off-diagonal tiles;
                        # accumulation order over J is free.  Scores run
                        # one J ahead of AV (st bufs=2) so ScalarE's exp
                        # paces the loop, never the PE.
                        order = list(range(4 * B, Jmax + 1)) + list(range(4 * B))
                        for n_j, J in enumerate(order):
                            st = stpool.tile([P, 512], f32, tag="st",
                                             name=f"st{J}_{h}_{B}")
                            nc.tensor.matmul(st[:],
                                             K_sb[:, h, P * J:P * (J + 1)],
                                             Q_sb[:, h, ib],
                                             start=True, stop=True)
                            if J >= 4 * B:
                                nc.vector.tensor_tensor(
                                    st[:], st[:], masks_sb[:, J % 4, :],
                                    Alu.add)
                            pt = ptpool.tile([P, 512], bf16, tag="pt",
                                             name=f"pt{J}_{h}_{B}")
                            nc.scalar.activation(pt[:], st[:], Act.Exp,
                                                 scale=inv_sqrt_dh)
                            pts[J] = pt
                            if n_j >= 1:
                                Jp = order[n_j - 1]
                                nc.tensor.matmul(av[:], V_sb[:, Jp, hs],
                                                 pts[Jp][:],
                                                 start=(n_j == 1),
                                                 stop=False)
                                nc.tensor.matmul(ssum, ones_bf[:],
                                                 pts[Jp][:],
                                                 start=(n_j == 1),
                                                 stop=False)
                                del pts[Jp]
                        Jl = order[-1]
                        nc.tensor.matmul(av[:], V_sb[:, Jl, hs],
                                         pts[Jl][:], start=(Jmax == 0),
                                         stop=True)
                        nc.tensor.matmul(ssum, ones_bf[:], pts[Jl][:],
                                         start=(Jmax == 0), stop=True)
                        rinv = finpool.tile([1, 512], f32, tag="rinv")
                        nc.vector.reciprocal_approx_fast(rinv[:], ssum)
                        rinvb = finpool.tile([1, 512], bf16, tag="rinvb")
                        nc.vector.tensor_copy(rinvb[:], rinv[:])
                        rb = ssumps.tile([P, 512], f32, tag="ss",
                                         name=f"rb{h}_{B}")
                        nc.tensor.matmul(rb[:], ones_row[:], rinvb[:],
                                         start=True, stop=True)
                        rbs = finpool.tile([P, 512], f32, tag="rbs")
                        nc.scalar.activation(rbs[:], rb[:], Act.Copy)
                        att = finpool.tile([P, 512], bf16, tag="att")
                        nc.vector.tensor_tensor(att[:], av[:], rbs[:], Alu.mult)
                        nc.scalar.dma_start(ag_in[B][DH * h:DH * (h + 1), :],
                                            att[:])
                    nc.gpsimd.collective_compute(
                        "AllGather", Alu.bypass,
                        replica_groups=[list(range(n_cores))],
                        ins=[ag_in[B][:].opt()],
                        outs=[ag_out[B][:].opt()])
                    # residual load (never gated) on sync; att^T readback
                    # (gated on the collective) on gpsimd
                    sl = slice(512 * B, 512 * B + 512)
                    xct = xcpool.tile([P, DL // P, 512], f32, tag="xct",
                                      name=f"xct{B}")
                    nc.scalar.dma_start(xct[:], xctr[:, :, sl])
                    xct_tiles[B] = xct
                    agt = agpool.tile([P, KC, 512], bf16, tag="ag",
                                      name=f"agt{B}")
                    agr = ag_out[B][:].rearrange("(kc p) t -> p kc t", p=P)
                    rb_eng = nc.sync if B % 2 == 0 else nc.gpsimd
                    for kq in range(0, KC, 4):
                        rb_eng.dma_start(agt[:, kq:kq + 4, :],
                                         agr[:, kq:kq + 4, :])
                    ag_tiles[B] = agt

                def phase4(B):
                    sl = slice(512 * B, 512 * B + 512)
                    om = [accpool.tile([P, 512], f32, tag="acc",
                                       name=f"om{js}_{B}")
                          for js in range(DL // P)]
                    for kc in range(KC):
                        for js in range(DL // P):
                            nc.tensor.matmul(
                                om[js][:], wo_sb[:, kc, P * js:P * (js + 1)],
                                ag_tiles[B][:, kc, :], start=(kc == 0),
                                stop=(kc == KC - 1))
                    for js in range(DL // P):
                        osb = opool.tile([P, 512], f32, tag="osb",
                                         name=f"osb{js}_{B}")
                        nc.vector.tensor_tensor(osb[:], om[js][:],
                                                xct_tiles[B][:, js, :], Alu.add)
                        nc.scalar.dma_start(outr[:, js, sl], osb[:])

                for B in range(TB):
                    phase1(B)
                    phase2(B)
                    if B >= 2:
                        phase4(B - 2)
                phase4(TB - 2)
                phase4(TB - 1)

    nc.compile()
    return nc


# --------------------------------------------------------------------------
# host-side prep / entry point
# --------------------------------------------------------------------------
def prepare_inputs(x, cos, sin, ln_w, Wq, Wk, Wv, Wo, n_cores, heads_per_core):
    import ml_dtypes
    bf16 = ml_dtypes.bfloat16
    DH = 128
    DL = heads_per_core * DH
    x = np.ascontiguousarray(np.asarray(x, dtype=np.float32))
    cos = np.asarray(cos, dtype=np.float32)
    sin = np.asarray(sin, dtype=np.float32)
    ln_w = np.ascontiguousarray(np.asarray(ln_w, dtype=np.float32))
    xT = np.ascontiguousarray(x.T.astype(bf16))
    cosT = np.ascontiguousarray(cos.T)
    sinT = np.ascontiguousarray(sin.T)
    R = np.zeros((DH, DH), dtype=np.float32)
    R[np.arange(64), np.arange(64) + 64] = -1.0
    R[np.arange(64) + 64, np.arange(64)] = 1.0
    rot_t = np.ascontiguousarray(R.T.astype(bf16))
    D = x.shape[1]
    KC = D // DH
    # fold ln_w into the projection weights (h = ln_w * x * r, so scale the
    # contract-dim rows of W.T by ln_w)
    lncol = ln_w[:, None]

    def pretile(wT):
        # (D, DL) -> SBUF layout [P, KC*DL]: element (p, kc, j) = wT[128 kc + p, j]
        return np.ascontiguousarray(
            wT.reshape(KC, DH, DL).transpose(1, 0, 2).reshape(DH, KC * DL)
            .astype(bf16))

    in_maps = []
    for c in range(n_cores):
        cols = slice(c * DL, (c + 1) * DL)
        in_maps.append({
            "xT": xT,
            "x_colsT": np.ascontiguousarray(x[:, cols].T),
            "wq_t": pretile(np.asarray(Wq, np.float32)[cols, :].T * lncol),
            "wk_t": pretile(np.asarray(Wk, np.float32)[cols, :].T * lncol),
            "wv_t": pretile(np.asarray(Wv, np.float32)[cols, :].T * lncol),
            "wo_t": pretile(np.asarray(Wo, np.float32)[cols, :].T),
            "cosT": cosT,
            "sinT": sinT,
            "rot_t": rot_t,
        })
    return in_maps


_NC_CACHE = {}


def kernel(x, cos, sin, attention_mask, ln_w, Wq, Wk, Wv, Wo,
           _trace=False, _trace_cores=None):
    from concourse.bass_utils import run_bass_kernel_spmd

    cfg = CFG_FULL
    key = tuple(sorted(cfg.items()))
    if key not in _NC_CACHE:
        _NC_CACHE[key] = build_nc(**cfg)
    nc = _NC_CACHE[key]
    n_cores = cfg["n_cores"]
    in_maps = prepare_inputs(x, cos, sin, ln_w, Wq, Wk, Wv, Wo,
                             n_cores, cfg["heads_per_core"])
    res = run_bass_kernel_spmd(nc, in_maps, core_ids=list(range(n_cores)),
                               trace=_trace, trace_cores=_trace_cores)
    out = np.concatenate(
        [res.results[c]["out_colsT"].T for c in range(n_cores)], axis=1)
    kernel.last_result = res
    return out
